# revision 3
# baseline (speedup 1.0000x reference)
"""LIF bank (nn_LIFBank_17059610100011) Trainium2 Bass kernel, v5.

The entire per-lane LIF recurrence (T sequential steps) runs INSIDE one
custom DVE instruction per 128-lane group: a hand-written 8-uop FSM
program streams time along the free dimension at 6 cycles/timestep,
keeping state (v, theta, refractory gates) in per-block a-flops and
emitting (s_t, v_t) interleaved. Replaces the v4 kernel's 6 DVE
instructions per timestep (6000 instructions/core -> 32).

Steady ring (one element per 6 cycles), phases = uops PH0..PH5:
  ph0 (consumes u): p=u*G2, m=p*G1, w=m+AV, s=(w>=TH)->b3; emits s
  ph1: q=s*TH, v'=w-q->b4; emits v'
  ph2: G2<-G1 (a1), G1'=1-s (a2), t1=TH*beta, t2=t1+c
  ph3: gs=s*gamma, AV'=v'*alpha (a3), TH'=t2+gs (a4)
  ph4, ph5: bubbles (dependence-loop spacing)
States: a1=G2, a2=G1, a3=alpha*v, a4=theta; swap2=gamma, swap3=alpha;
beta/c/1.0 ride ph2 delay lanes from CONST_1/CONST_2/ONE_F32.
Rounding order matches the jax reference exactly (same op/rounding
sequence as the v4 kernel; bit-identical results).
"""

import numpy as np

ALPHA = 0.95
BETA = 0.995
GAMMA = 0.35

B, N, T = 16, 2048, 1000
NCORES = 8
NSH = N // NCORES           # 256 neurons/core
NG = (B * NSH) // 128       # 32 groups of 128 lanes/core

_CACHE = {}

# ---------------------------------------------------------------- schedule --
# Engine-neutral description of the uop program (validated cycle-accurately
# offline against the numpy reference before HW bring-up).

PREV_ALU, CURR_ALU, NEXT_A, NEXT_B, CURR_SWAP = 0, 1, 2, 3, 4
PD0, PD1, PD2, PD3, PD4, PD5 = 5, 6, 7, 8, 9, 10
D_PREV_ALU, D_PREV_DELAY = 0, 5
I_SRC0, I_SRC1, I_C0, I_C1, I_C2, I_ZERO, I_ONE = (
    "SRC0", "SRC1", "C0", "C1", "C2", "ZERO", "ONE")


class _Blk:
    def __init__(self, op="NOP", a_src=PREV_ALU, b_src=PREV_ALU, out_en=False,
                 afl=False, bfl=False, swap_en=False, delay=None):
        self.op, self.a_src, self.b_src = op, a_src, b_src
        self.out_en, self.afl, self.bfl, self.swap_en = out_en, afl, bfl, swap_en
        self.delay = delay or {}


class _Uop:
    def __init__(self, name="", req0=False, req1=False, wr_en=False,
                 out_sel="ALU", trig=None, repeat=1):
        self.name, self.req0, self.req1 = name, req0, req1
        self.wr_en, self.out_sel = wr_en, out_sel
        self.trig, self.repeat = trig or [], repeat
        self.inp = {}
        self.blocks = [_Blk() for _ in range(8)]


def _build_sched():
    init0 = _Uop("INIT0", req1=True, trig=[("COUNT", 1)])
    init0.inp = {1: I_SRC1, 2: I_C0, 3: I_ZERO, 4: I_ONE}
    init0.blocks[0] = _Blk(delay={0: D_PREV_DELAY, 1: D_PREV_DELAY,
                                  2: D_PREV_DELAY, 3: D_PREV_DELAY})
    init0.blocks[1] = _Blk("BYPASS", PD3, PD3, afl=True,
                           delay={0: D_PREV_DELAY, 1: D_PREV_DELAY,
                                  2: D_PREV_DELAY, 3: D_PREV_DELAY})
    init0.blocks[2] = _Blk("BYPASS", PD3, PD1, afl=True, swap_en=True,
                           delay={0: D_PREV_DELAY, 2: D_PREV_DELAY,
                                  3: D_PREV_DELAY})
    init0.blocks[3] = _Blk("BYPASS", PD2, PD0, afl=True, swap_en=True,
                           delay={3: D_PREV_DELAY})
    init0.blocks[4] = _Blk("BYPASS", PD3, PD3, afl=True)

    init1 = _Uop("INIT1", trig=[("COUNT", 2)])

    ph0 = _Uop("PH0", req0=True, wr_en=True, out_sel="D2", trig=[("COUNT", 3)])
    ph0.inp = {0: I_SRC0}
    ph0.blocks[0] = _Blk("MUL", PREV_ALU, NEXT_A, out_en=True)
    ph0.blocks[1] = _Blk("MUL", PREV_ALU, NEXT_A, out_en=True)
    ph0.blocks[2] = _Blk("ADD", PREV_ALU, NEXT_A, out_en=True)
    ph0.blocks[3] = _Blk("IS_GE", PREV_ALU, NEXT_A, out_en=True, bfl=True)
    ph0.blocks[4] = _Blk(delay={2: D_PREV_ALU})
    ph0.blocks[5] = _Blk(delay={2: D_PREV_DELAY})
    ph0.blocks[6] = _Blk(delay={2: D_PREV_DELAY})
    ph0.blocks[7] = _Blk(delay={2: D_PREV_DELAY})

    ph1 = _Uop("PH1", wr_en=True, out_sel="ALU", trig=[("COUNT", 4)])
    ph1.blocks[3] = _Blk("MUL", CURR_ALU, NEXT_A, out_en=True,
                         delay={0: D_PREV_ALU})
    ph1.blocks[4] = _Blk("SUB", PD0, PREV_ALU, out_en=True, bfl=True)
    ph1.blocks[5] = _Blk("BYPASS", PREV_ALU, PREV_ALU, out_en=True)
    ph1.blocks[6] = _Blk("BYPASS", PREV_ALU, PREV_ALU, out_en=True)
    ph1.blocks[7] = _Blk("BYPASS", PREV_ALU, PREV_ALU, out_en=True)

    ph2 = _Uop("PH2", trig=[("COUNT", 5)])
    ph2.inp = {2: I_ONE, 5: I_C1, 6: I_C2}
    ph2.blocks[0] = _Blk(delay={1: D_PREV_DELAY, 4: D_PREV_DELAY,
                                5: D_PREV_DELAY})
    ph2.blocks[1] = _Blk("BYPASS", NEXT_A, NEXT_A, afl=True,
                         delay={1: D_PREV_DELAY, 4: D_PREV_DELAY,
                                5: D_PREV_DELAY})
    ph2.blocks[2] = _Blk("SUB", PD1, NEXT_B, afl=True,
                         delay={4: D_PREV_DELAY, 5: D_PREV_DELAY})
    ph2.blocks[3] = _Blk("MUL", NEXT_A, PD4, out_en=True,
                         delay={5: D_PREV_DELAY})
    ph2.blocks[4] = _Blk("ADD", PREV_ALU, PD5, out_en=True)

    ph3 = _Uop("PH3", trig=[("COUNT", 6)])
    ph3.blocks[2] = _Blk("MUL", NEXT_B, CURR_SWAP, out_en=True)
    ph3.blocks[3] = _Blk("MUL", NEXT_B, CURR_SWAP, afl=True,
                         delay={3: D_PREV_ALU})
    ph3.blocks[4] = _Blk("ADD", CURR_ALU, PD3, afl=True)

    ph4 = _Uop("PH4", trig=[("SRC_DONE", "IDLE"), ("COUNT", 2)])

    return [init0, init1, ph0, ph1, ph2, ph3, ph4]


# ------------------------------------------------------------- registration --

def _register_op():
    import concourse.dve_ops as dvo
    from concourse.dve_spec import Spec, Src0, Src1, C0, C1, C2
    from concourse.dve_uop import (
        UopConfig, UopDpConfig, DveOpSpec, AluOp, AluInp, DelayInp, InpSel,
        OutSel, OutPath, Trigger, ENABLE, DISABLE,
    )

    name = "LIF_SCAN"
    if name in dvo._SUB_OPCODE_FOR_NAME:
        return next(o for o in dvo.OPS if o.name == name)

    alu = {"BYPASS": AluOp.BYPASS, "ADD": AluOp.ADD, "SUB": AluOp.SUBTRACT,
           "MUL": AluOp.MULTIPLY, "IS_GE": AluOp.IS_GE}
    ain = {PREV_ALU: AluInp.PREV_ALU_OUT, CURR_ALU: AluInp.CURR_ALU_OUT,
           NEXT_A: AluInp.NEXT_ALU_OUT_A, NEXT_B: AluInp.NEXT_ALU_OUT_B,
           CURR_SWAP: AluInp.CURR_SWAP_OUT,
           PD0: AluInp.PREV_DELAY_0, PD1: AluInp.PREV_DELAY_1,
           PD2: AluInp.PREV_DELAY_2, PD3: AluInp.PREV_DELAY_3,
           PD4: AluInp.PREV_DELAY_4, PD5: AluInp.PREV_DELAY_5}
    din = {D_PREV_ALU: DelayInp.PREV_ALU_OUT, D_PREV_DELAY: DelayInp.PREV_DELAY}
    isel = {I_SRC0: InpSel.SRC_0, I_SRC1: InpSel.SRC_1, I_C0: InpSel.CONST_0,
            I_C1: InpSel.CONST_1, I_C2: InpSel.CONST_2, I_ZERO: InpSel.ZERO,
            I_ONE: InpSel.ONE_F32}
    osel = {"ALU": OutSel.ALU_OUT, "D2": OutSel.DELAY_2}
    tmap = {"COUNT": Trigger.COUNT, "SRC_DONE": Trigger.SRC_TENSOR_DONE}

    uops = []
    for su in _build_sched():
        u = UopConfig()
        for lane, sel in su.inp.items():
            u.enable_input(isel[sel], lane)
        u.require_inp0 = ENABLE if su.req0 else DISABLE
        u.require_inp1 = ENABLE if su.req1 else DISABLE
        if su.wr_en:
            u.out[OutPath.WR0_LO] = osel[su.out_sel]
            u.out_enable[OutPath.WR0_LO] = ENABLE
        trigs, nexts = [], []
        for kind, nxt in su.trig:
            trigs.append(tmap[kind])
            nexts.append(0 if nxt == "IDLE" else nxt)
        while len(trigs) < 3:
            trigs.append(Trigger.NONE)
            nexts.append(0)
        u.trigger = tuple(trigs)
        u.next_uop = tuple(nexts)
        u.repeat_count = su.repeat
        for k, sb in enumerate(su.blocks):
            d = UopDpConfig()
            if sb.op != "NOP":
                d.op = alu[sb.op]
                d.alu_src0 = ain[sb.a_src]
                d.alu_src1 = ain[sb.b_src]
                d.alu_out_enable = ENABLE if sb.out_en else DISABLE
                d.alu_out_a_enable = ENABLE if sb.afl else DISABLE
                d.alu_out_b_enable = ENABLE if sb.bfl else DISABLE
                d.swap_enable = ENABLE if sb.swap_en else DISABLE
            for lane, src in sb.delay.items():
                d.delay[lane] = din[src]
                d.delay_enable[lane] = ENABLE
            u.datapath_config[k] = d
        uops.append(u)

    opcode = dvo._CUSTOM_DVE_ROW_BASE + len(dvo.OPS)
    spec = Spec(body=(Src0 * C0 + C1 * C2) * Src1, reference=_lif_reference)
    hand = DveOpSpec(name=name, opcode=opcode, uops=uops, rd1_en=True)
    hand.validate("v3")
    op = dvo.DveOp(name, spec, subdim=False, uops_sha={"v3": hand.sha("v3")})
    dvo._SUB_OPCODE_FOR_NAME[name] = opcode
    dvo.OPS.append(op)
    dvo.CUSTOM_DVE_SPECS[name] = spec
    dvo._COMPILE_CACHE[(name, "v3")] = hand
    return op


def _lif_reference(in0, in1, c0, c1, c2):
    """CoreSim reference: in0 [P,T] u; out [P,2T] interleaved (s, v)."""
    F = np.float32
    in0 = np.asarray(in0, F)
    P = in0.shape[0]
    Tn = int(np.prod(in0.shape[1:]))
    u = in0.reshape(P, Tn)
    alpha = (np.asarray(in1, F).reshape(P)[:, None]
             if in1 is not None else np.full((P, 1), 0.95, F))
    gamma = F(np.asarray(c0, F).flat[0] if isinstance(c0, np.ndarray) else c0)
    beta = F(np.asarray(c1, F).flat[0] if isinstance(c1, np.ndarray) else c1)
    c = F(c2)
    v = np.zeros((P, 1), F)
    th = np.ones((P, 1), F)
    g1 = np.ones((P, 1), F)
    g2 = np.ones((P, 1), F)
    out = np.empty((P, Tn, 2), F)
    for t in range(Tn):
        p = (u[:, t:t + 1] * g2).astype(F)
        m = (p * g1).astype(F)
        w = ((alpha * v).astype(F) + m).astype(F)
        s = (w >= th).astype(F)
        q = (s * th).astype(F)
        vn = (w - q).astype(F)
        t1 = (th * beta).astype(F)
        t2 = (t1 + c).astype(F)
        gs = (s * gamma).astype(F)
        th = (t2 + gs).astype(F)
        out[:, t, 0] = s[:, 0]
        out[:, t, 1] = vn[:, 0]
        g2 = g1
        g1 = (F(1.0) - s).astype(F)
        v = vn
    return out.reshape(P, 2 * Tn)


# ------------------------------------------------------------------ kernel --

def _build_nc(c_imm):
    import concourse.bacc as bacc
    import concourse.mybir as mybir
    import concourse.tile as tile

    LIF = _register_op()
    f32 = mybir.dt.float32

    bf16 = mybir.dt.bfloat16
    nc = bacc.Bacc("TRN2", target_bir_lowering=False, num_devices=NCORES)
    u_d = nc.dram_tensor("u", [NG, 128, T], f32, kind="ExternalInput")
    sv_d = nc.dram_tensor("sv", [NG, 128, 2 * T], bf16, kind="ExternalOutput")
    vec = nc.vector

    with tile.TileContext(nc) as tc_ctx:
        with (
            tc_ctx.tile_pool(name="state", bufs=1) as st,
            tc_ctx.tile_pool(name="upool", bufs=NG) as up,
            tc_ctx.tile_pool(name="svpool", bufs=12) as svp,
        ):
            al = st.tile([128, 1], f32, tag="alpha", name="alpha")
            vec.memset(al[:], ALPHA)
            # preload ALL input tiles up front (128KB/partition, fits SBUF):
            # the DVE never waits on an input DMA after the first group.
            uts = []
            for g in range(NG):
                ut = up.tile([128, T], f32, tag="u", name=f"u{g}")
                if g == 0:
                    # split the first load so LIF #0's dependency clears
                    # sooner (two queues instead of one)
                    h = T // 2
                    nc.sync.dma_start(ut[:, :h], u_d[g, :, :h])
                    nc.sync.dma_start(ut[:, h:], u_d[g, :, h:])
                else:
                    nc.sync.dma_start(ut[:], u_d[g, :, :])
                uts.append(ut)
            for g in range(NG):
                ut = uts[g]
                svt = svp.tile([128, 2 * T], bf16, tag="sv", name=f"sv{g}")
                vec._custom_dve(
                    LIF, out=svt[:], in0=ut[:], in1=al[:],
                    s0=GAMMA, s1=BETA, imm2=c_imm,
                )
                # split the store across queues to spread DMA bandwidth;
                # the final group is the only store on the critical tail
                # path, so split it finer for more queue parallelism
                nsplit = 8 if g == NG - 1 else 4
                q = (2 * T) // nsplit
                for j in range(nsplit):
                    nc.sync.dma_start(sv_d[g, :, j * q:(j + 1) * q],
                                      svt[:, j * q:(j + 1) * q])

    nc.compile()
    return nc


def _get_nc(c_imm):
    key = float(c_imm)
    if key not in _CACHE:
        _CACHE[key] = _build_nc(key)
    return _CACHE[key]


def _shard_inputs(u):
    u = np.asarray(u, dtype=np.float32)
    in_maps = []
    for c in range(NCORES):
        lo, hi = c * NSH, (c + 1) * NSH
        uc = np.ascontiguousarray(
            u[:, lo:hi, :].reshape(B * NSH, T).reshape(NG, 128, T))
        in_maps.append({"u": uc})
    return in_maps


def _unshard(res):
    s_full = np.empty((B, N, T), dtype=np.float32)
    v_full = np.empty((B, N, T), dtype=np.float32)
    for c in range(NCORES):
        lo, hi = c * NSH, (c + 1) * NSH
        sv = np.asarray(res[c]["sv"]).astype(np.float32).reshape(B * NSH, T, 2)
        s_full[:, lo:hi, :] = sv[:, :, 0].reshape(B, NSH, T)
        v_full[:, lo:hi, :] = sv[:, :, 1].reshape(B, NSH, T)
    return s_full, v_full


def _host_fallback(u, theta_base):
    """Exact numpy simulation; only used if theta_base is non-uniform."""
    u = np.asarray(u, np.float32)
    b, n, t = u.shape
    tb = np.asarray(theta_base, np.float32)[0, :, 0]
    v = np.zeros((b, n), np.float32)
    theta = np.broadcast_to(tb, (b, n)).astype(np.float32).copy()
    ref = np.zeros((b, n), np.float32)
    c = (tb * np.float32(1.0 - BETA)).astype(np.float32)
    ss = np.empty((b, n, t), np.float32)
    vs = np.empty((b, n, t), np.float32)
    for i in range(t):
        u_eff = np.where(ref > 0, np.float32(0.0), u[:, :, i])
        v = (np.float32(ALPHA) * v + u_eff).astype(np.float32)
        s = (v >= theta).astype(np.float32)
        v = (v - s * theta).astype(np.float32)
        ref = np.where(s > 0, np.float32(2.0),
                       np.maximum(ref - 1.0, 0.0).astype(np.float32))
        theta = ((theta * np.float32(BETA) + c)
                 + np.float32(GAMMA) * s).astype(np.float32)
        ss[:, :, i] = s
        vs[:, :, i] = v
    return ss, vs


def run(u, theta_base, trace=False):
    from concourse.bass_utils import run_bass_kernel_spmd

    tb = np.asarray(theta_base, dtype=np.float32)
    c_imm = float(np.float32(tb.flat[0]) * np.float32(1.0 - BETA))
    nc = _get_nc(c_imm)
    in_maps = _shard_inputs(u)
    res = run_bass_kernel_spmd(nc, in_maps, core_ids=list(range(NCORES)),
                               trace=trace)
    s_full, v_full = _unshard(res.results)
    return (s_full, v_full), res


def kernel(u, theta_base):
    tb = np.asarray(theta_base, dtype=np.float32)
    if not np.all(tb == tb.flat[0]):
        return _host_fallback(u, theta_base)
    (s_full, v_full), _ = run(u, theta_base)
    return s_full, v_full



# revision 5
# speedup vs baseline: 1.0059x; 1.0059x over previous
"""LIF bank (nn_LIFBank_17059610100011) Trainium2 Bass kernel, v5.

The entire per-lane LIF recurrence (T sequential steps) runs INSIDE one
custom DVE instruction per 128-lane group: a hand-written 8-uop FSM
program streams time along the free dimension at 6 cycles/timestep,
keeping state (v, theta, refractory gates) in per-block a-flops and
emitting (s_t, v_t) interleaved. Replaces the v4 kernel's 6 DVE
instructions per timestep (6000 instructions/core -> 32).

Steady ring (one element per 6 cycles), phases = uops PH0..PH5:
  ph0 (consumes u): p=u*G2, m=p*G1, w=m+AV, s=(w>=TH)->b3; emits s
  ph1: q=s*TH, v'=w-q->b4; emits v'
  ph2: G2<-G1 (a1), G1'=1-s (a2), t1=TH*beta, t2=t1+c
  ph3: gs=s*gamma, AV'=v'*alpha (a3), TH'=t2+gs (a4)
  ph4, ph5: bubbles (dependence-loop spacing)
States: a1=G2, a2=G1, a3=alpha*v, a4=theta; swap2=gamma, swap3=alpha;
beta/c/1.0 ride ph2 delay lanes from CONST_1/CONST_2/ONE_F32.
Rounding order matches the jax reference exactly (same op/rounding
sequence as the v4 kernel; bit-identical results).
"""

import numpy as np

ALPHA = 0.95
BETA = 0.995
GAMMA = 0.35

B, N, T = 16, 2048, 1000
NCORES = 8
NSH = N // NCORES           # 256 neurons/core
NG = (B * NSH) // 128       # 32 groups of 128 lanes/core

_CACHE = {}

# ---------------------------------------------------------------- schedule --
# Engine-neutral description of the uop program (validated cycle-accurately
# offline against the numpy reference before HW bring-up).

PREV_ALU, CURR_ALU, NEXT_A, NEXT_B, CURR_SWAP = 0, 1, 2, 3, 4
PD0, PD1, PD2, PD3, PD4, PD5 = 5, 6, 7, 8, 9, 10
D_PREV_ALU, D_PREV_DELAY = 0, 5
I_SRC0, I_SRC1, I_C0, I_C1, I_C2, I_ZERO, I_ONE = (
    "SRC0", "SRC1", "C0", "C1", "C2", "ZERO", "ONE")


class _Blk:
    def __init__(self, op="NOP", a_src=PREV_ALU, b_src=PREV_ALU, out_en=False,
                 afl=False, bfl=False, swap_en=False, delay=None):
        self.op, self.a_src, self.b_src = op, a_src, b_src
        self.out_en, self.afl, self.bfl, self.swap_en = out_en, afl, bfl, swap_en
        self.delay = delay or {}


class _Uop:
    def __init__(self, name="", req0=False, req1=False, wr_en=False,
                 out_sel="ALU", trig=None, repeat=1):
        self.name, self.req0, self.req1 = name, req0, req1
        self.wr_en, self.out_sel = wr_en, out_sel
        self.trig, self.repeat = trig or [], repeat
        self.inp = {}
        self.blocks = [_Blk() for _ in range(8)]


def _build_sched():
    init0 = _Uop("INIT0", req1=True, trig=[("COUNT", 1)])
    init0.inp = {1: I_SRC1, 2: I_C0, 3: I_ZERO, 4: I_ONE}
    init0.blocks[0] = _Blk(delay={0: D_PREV_DELAY, 1: D_PREV_DELAY,
                                  2: D_PREV_DELAY, 3: D_PREV_DELAY})
    init0.blocks[1] = _Blk("BYPASS", PD3, PD3, afl=True,
                           delay={0: D_PREV_DELAY, 1: D_PREV_DELAY,
                                  2: D_PREV_DELAY, 3: D_PREV_DELAY})
    init0.blocks[2] = _Blk("BYPASS", PD3, PD1, afl=True, swap_en=True,
                           delay={0: D_PREV_DELAY, 2: D_PREV_DELAY,
                                  3: D_PREV_DELAY})
    init0.blocks[3] = _Blk("BYPASS", PD2, PD0, afl=True, swap_en=True,
                           delay={3: D_PREV_DELAY})
    init0.blocks[4] = _Blk("BYPASS", PD3, PD3, afl=True)

    init1 = _Uop("INIT1", trig=[("COUNT", 2)])

    ph0 = _Uop("PH0", req0=True, wr_en=True, out_sel="D2", trig=[("COUNT", 3)])
    ph0.inp = {0: I_SRC0}
    ph0.blocks[0] = _Blk("MUL", PREV_ALU, NEXT_A, out_en=True)
    ph0.blocks[1] = _Blk("MUL", PREV_ALU, NEXT_A, out_en=True)
    ph0.blocks[2] = _Blk("ADD", PREV_ALU, NEXT_A, out_en=True)
    ph0.blocks[3] = _Blk("IS_GE", PREV_ALU, NEXT_A, out_en=True, bfl=True)
    ph0.blocks[4] = _Blk(delay={2: D_PREV_ALU})
    ph0.blocks[5] = _Blk(delay={2: D_PREV_DELAY})
    ph0.blocks[6] = _Blk(delay={2: D_PREV_DELAY})
    ph0.blocks[7] = _Blk(delay={2: D_PREV_DELAY})

    ph1 = _Uop("PH1", wr_en=True, out_sel="ALU", trig=[("COUNT", 4)])
    ph1.blocks[3] = _Blk("MUL", CURR_ALU, NEXT_A, out_en=True,
                         delay={0: D_PREV_ALU})
    ph1.blocks[4] = _Blk("SUB", PD0, PREV_ALU, out_en=True, bfl=True)
    ph1.blocks[5] = _Blk("BYPASS", PREV_ALU, PREV_ALU, out_en=True)
    ph1.blocks[6] = _Blk("BYPASS", PREV_ALU, PREV_ALU, out_en=True)
    ph1.blocks[7] = _Blk("BYPASS", PREV_ALU, PREV_ALU, out_en=True)

    ph2 = _Uop("PH2", trig=[("COUNT", 5)])
    ph2.inp = {2: I_ONE, 5: I_C1, 6: I_C2}
    ph2.blocks[0] = _Blk(delay={1: D_PREV_DELAY, 4: D_PREV_DELAY,
                                5: D_PREV_DELAY})
    ph2.blocks[1] = _Blk("BYPASS", NEXT_A, NEXT_A, afl=True,
                         delay={1: D_PREV_DELAY, 4: D_PREV_DELAY,
                                5: D_PREV_DELAY})
    ph2.blocks[2] = _Blk("SUB", PD1, NEXT_B, afl=True,
                         delay={4: D_PREV_DELAY, 5: D_PREV_DELAY})
    ph2.blocks[3] = _Blk("MUL", NEXT_A, PD4, out_en=True,
                         delay={5: D_PREV_DELAY})
    ph2.blocks[4] = _Blk("ADD", PREV_ALU, PD5, out_en=True)

    ph3 = _Uop("PH3", trig=[("COUNT", 6)])
    ph3.blocks[2] = _Blk("MUL", NEXT_B, CURR_SWAP, out_en=True)
    ph3.blocks[3] = _Blk("MUL", NEXT_B, CURR_SWAP, afl=True,
                         delay={3: D_PREV_ALU})
    ph3.blocks[4] = _Blk("ADD", CURR_ALU, PD3, afl=True)

    ph4 = _Uop("PH4", trig=[("SRC_DONE", "IDLE"), ("COUNT", 2)])

    return [init0, init1, ph0, ph1, ph2, ph3, ph4]


# ------------------------------------------------------------- registration --

def _register_op():
    import concourse.dve_ops as dvo
    from concourse.dve_spec import Spec, Src0, Src1, C0, C1, C2
    from concourse.dve_uop import (
        UopConfig, UopDpConfig, DveOpSpec, AluOp, AluInp, DelayInp, InpSel,
        OutSel, OutPath, Trigger, ENABLE, DISABLE,
    )

    name = "LIF_SCAN"
    if name in dvo._SUB_OPCODE_FOR_NAME:
        return next(o for o in dvo.OPS if o.name == name)

    alu = {"BYPASS": AluOp.BYPASS, "ADD": AluOp.ADD, "SUB": AluOp.SUBTRACT,
           "MUL": AluOp.MULTIPLY, "IS_GE": AluOp.IS_GE}
    ain = {PREV_ALU: AluInp.PREV_ALU_OUT, CURR_ALU: AluInp.CURR_ALU_OUT,
           NEXT_A: AluInp.NEXT_ALU_OUT_A, NEXT_B: AluInp.NEXT_ALU_OUT_B,
           CURR_SWAP: AluInp.CURR_SWAP_OUT,
           PD0: AluInp.PREV_DELAY_0, PD1: AluInp.PREV_DELAY_1,
           PD2: AluInp.PREV_DELAY_2, PD3: AluInp.PREV_DELAY_3,
           PD4: AluInp.PREV_DELAY_4, PD5: AluInp.PREV_DELAY_5}
    din = {D_PREV_ALU: DelayInp.PREV_ALU_OUT, D_PREV_DELAY: DelayInp.PREV_DELAY}
    isel = {I_SRC0: InpSel.SRC_0, I_SRC1: InpSel.SRC_1, I_C0: InpSel.CONST_0,
            I_C1: InpSel.CONST_1, I_C2: InpSel.CONST_2, I_ZERO: InpSel.ZERO,
            I_ONE: InpSel.ONE_F32}
    osel = {"ALU": OutSel.ALU_OUT, "D2": OutSel.DELAY_2}
    tmap = {"COUNT": Trigger.COUNT, "SRC_DONE": Trigger.SRC_TENSOR_DONE}

    uops = []
    for su in _build_sched():
        u = UopConfig()
        for lane, sel in su.inp.items():
            u.enable_input(isel[sel], lane)
        u.require_inp0 = ENABLE if su.req0 else DISABLE
        u.require_inp1 = ENABLE if su.req1 else DISABLE
        if su.wr_en:
            u.out[OutPath.WR0_LO] = osel[su.out_sel]
            u.out_enable[OutPath.WR0_LO] = ENABLE
        trigs, nexts = [], []
        for kind, nxt in su.trig:
            trigs.append(tmap[kind])
            nexts.append(0 if nxt == "IDLE" else nxt)
        while len(trigs) < 3:
            trigs.append(Trigger.NONE)
            nexts.append(0)
        u.trigger = tuple(trigs)
        u.next_uop = tuple(nexts)
        u.repeat_count = su.repeat
        for k, sb in enumerate(su.blocks):
            d = UopDpConfig()
            if sb.op != "NOP":
                d.op = alu[sb.op]
                d.alu_src0 = ain[sb.a_src]
                d.alu_src1 = ain[sb.b_src]
                d.alu_out_enable = ENABLE if sb.out_en else DISABLE
                d.alu_out_a_enable = ENABLE if sb.afl else DISABLE
                d.alu_out_b_enable = ENABLE if sb.bfl else DISABLE
                d.swap_enable = ENABLE if sb.swap_en else DISABLE
            for lane, src in sb.delay.items():
                d.delay[lane] = din[src]
                d.delay_enable[lane] = ENABLE
            u.datapath_config[k] = d
        uops.append(u)

    opcode = dvo._CUSTOM_DVE_ROW_BASE + len(dvo.OPS)
    spec = Spec(body=(Src0 * C0 + C1 * C2) * Src1, reference=_lif_reference)
    hand = DveOpSpec(name=name, opcode=opcode, uops=uops, rd1_en=True)
    hand.validate("v3")
    op = dvo.DveOp(name, spec, subdim=False, uops_sha={"v3": hand.sha("v3")})
    dvo._SUB_OPCODE_FOR_NAME[name] = opcode
    dvo.OPS.append(op)
    dvo.CUSTOM_DVE_SPECS[name] = spec
    dvo._COMPILE_CACHE[(name, "v3")] = hand
    return op


def _lif_reference(in0, in1, c0, c1, c2):
    """CoreSim reference: in0 [P,T] u; out [P,2T] interleaved (s, v)."""
    F = np.float32
    in0 = np.asarray(in0, F)
    P = in0.shape[0]
    Tn = int(np.prod(in0.shape[1:]))
    u = in0.reshape(P, Tn)
    alpha = (np.asarray(in1, F).reshape(P)[:, None]
             if in1 is not None else np.full((P, 1), 0.95, F))
    gamma = F(np.asarray(c0, F).flat[0] if isinstance(c0, np.ndarray) else c0)
    beta = F(np.asarray(c1, F).flat[0] if isinstance(c1, np.ndarray) else c1)
    c = F(c2)
    v = np.zeros((P, 1), F)
    th = np.ones((P, 1), F)
    g1 = np.ones((P, 1), F)
    g2 = np.ones((P, 1), F)
    out = np.empty((P, Tn, 2), F)
    for t in range(Tn):
        p = (u[:, t:t + 1] * g2).astype(F)
        m = (p * g1).astype(F)
        w = ((alpha * v).astype(F) + m).astype(F)
        s = (w >= th).astype(F)
        q = (s * th).astype(F)
        vn = (w - q).astype(F)
        t1 = (th * beta).astype(F)
        t2 = (t1 + c).astype(F)
        gs = (s * gamma).astype(F)
        th = (t2 + gs).astype(F)
        out[:, t, 0] = s[:, 0]
        out[:, t, 1] = vn[:, 0]
        g2 = g1
        g1 = (F(1.0) - s).astype(F)
        v = vn
    return out.reshape(P, 2 * Tn)


# ------------------------------------------------------------------ kernel --

def _build_nc(c_imm):
    import concourse.bacc as bacc
    import concourse.mybir as mybir
    import concourse.tile as tile

    LIF = _register_op()
    f32 = mybir.dt.float32

    bf16 = mybir.dt.bfloat16
    nc = bacc.Bacc("TRN2", target_bir_lowering=False, num_devices=NCORES)
    u_d = nc.dram_tensor("u", [NG, 128, T], f32, kind="ExternalInput")
    sv_d = nc.dram_tensor("sv", [NG, 128, 2 * T], bf16, kind="ExternalOutput")
    vec = nc.vector

    with tile.TileContext(nc) as tc_ctx:
        with (
            tc_ctx.tile_pool(name="state", bufs=1) as st,
            tc_ctx.tile_pool(name="upool", bufs=NG) as up,
            tc_ctx.tile_pool(name="svpool", bufs=12) as svp,
        ):
            al = st.tile([128, 1], f32, tag="alpha", name="alpha")
            vec.memset(al[:], ALPHA)
            # preload ALL input tiles up front (128KB/partition, fits SBUF):
            # the DVE never waits on an input DMA after the first group.
            uts = []
            for g in range(NG):
                ut = up.tile([128, T], f32, tag="u", name=f"u{g}")
                nc.sync.dma_start(ut[:], u_d[g, :, :])
                uts.append(ut)
            for g in range(NG):
                ut = uts[g]
                svt = svp.tile([128, 2 * T], bf16, tag="sv", name=f"sv{g}")
                vec._custom_dve(
                    LIF, out=svt[:], in0=ut[:], in1=al[:],
                    s0=GAMMA, s1=BETA, imm2=c_imm,
                )
                # split the store across queues to spread DMA bandwidth;
                # the final group's store is the only one on the critical
                # tail path, so split it finer for more queue parallelism
                nsplit = 8 if g == NG - 1 else 4
                q = (2 * T) // nsplit
                for j in range(nsplit):
                    nc.sync.dma_start(sv_d[g, :, j * q:(j + 1) * q],
                                      svt[:, j * q:(j + 1) * q])

    nc.compile()
    return nc


def _get_nc(c_imm):
    key = float(c_imm)
    if key not in _CACHE:
        _CACHE[key] = _build_nc(key)
    return _CACHE[key]


def _shard_inputs(u):
    u = np.asarray(u, dtype=np.float32)
    in_maps = []
    for c in range(NCORES):
        lo, hi = c * NSH, (c + 1) * NSH
        uc = np.ascontiguousarray(
            u[:, lo:hi, :].reshape(B * NSH, T).reshape(NG, 128, T))
        in_maps.append({"u": uc})
    return in_maps


def _unshard(res):
    s_full = np.empty((B, N, T), dtype=np.float32)
    v_full = np.empty((B, N, T), dtype=np.float32)
    for c in range(NCORES):
        lo, hi = c * NSH, (c + 1) * NSH
        sv = np.asarray(res[c]["sv"]).astype(np.float32).reshape(B * NSH, T, 2)
        s_full[:, lo:hi, :] = sv[:, :, 0].reshape(B, NSH, T)
        v_full[:, lo:hi, :] = sv[:, :, 1].reshape(B, NSH, T)
    return s_full, v_full


def _host_fallback(u, theta_base):
    """Exact numpy simulation; only used if theta_base is non-uniform."""
    u = np.asarray(u, np.float32)
    b, n, t = u.shape
    tb = np.asarray(theta_base, np.float32)[0, :, 0]
    v = np.zeros((b, n), np.float32)
    theta = np.broadcast_to(tb, (b, n)).astype(np.float32).copy()
    ref = np.zeros((b, n), np.float32)
    c = (tb * np.float32(1.0 - BETA)).astype(np.float32)
    ss = np.empty((b, n, t), np.float32)
    vs = np.empty((b, n, t), np.float32)
    for i in range(t):
        u_eff = np.where(ref > 0, np.float32(0.0), u[:, :, i])
        v = (np.float32(ALPHA) * v + u_eff).astype(np.float32)
        s = (v >= theta).astype(np.float32)
        v = (v - s * theta).astype(np.float32)
        ref = np.where(s > 0, np.float32(2.0),
                       np.maximum(ref - 1.0, 0.0).astype(np.float32))
        theta = ((theta * np.float32(BETA) + c)
                 + np.float32(GAMMA) * s).astype(np.float32)
        ss[:, :, i] = s
        vs[:, :, i] = v
    return ss, vs


def run(u, theta_base, trace=False):
    from concourse.bass_utils import run_bass_kernel_spmd

    tb = np.asarray(theta_base, dtype=np.float32)
    c_imm = float(np.float32(tb.flat[0]) * np.float32(1.0 - BETA))
    nc = _get_nc(c_imm)
    in_maps = _shard_inputs(u)
    res = run_bass_kernel_spmd(nc, in_maps, core_ids=list(range(NCORES)),
                               trace=trace)
    s_full, v_full = _unshard(res.results)
    return (s_full, v_full), res


def kernel(u, theta_base):
    tb = np.asarray(theta_base, dtype=np.float32)
    if not np.all(tb == tb.flat[0]):
        return _host_fallback(u, theta_base)
    (s_full, v_full), _ = run(u, theta_base)
    return s_full, v_full



# revision 7
# speedup vs baseline: 1.0211x; 1.0151x over previous
"""LIF bank (nn_LIFBank_17059610100011) Trainium2 Bass kernel, v5.

The entire per-lane LIF recurrence (T sequential steps) runs INSIDE one
custom DVE instruction per 128-lane group: a hand-written 8-uop FSM
program streams time along the free dimension at 6 cycles/timestep,
keeping state (v, theta, refractory gates) in per-block a-flops and
emitting (s_t, v_t) interleaved. Replaces the v4 kernel's 6 DVE
instructions per timestep (6000 instructions/core -> 32).

Steady ring (one element per 6 cycles), phases = uops PH0..PH5:
  ph0 (consumes u): p=u*G2, m=p*G1, w=m+AV, s=(w>=TH)->b3; emits s
  ph1: q=s*TH, v'=w-q->b4; emits v'
  ph2: G2<-G1 (a1), G1'=1-s (a2), t1=TH*beta, t2=t1+c
  ph3: gs=s*gamma, AV'=v'*alpha (a3), TH'=t2+gs (a4)
  ph4, ph5: bubbles (dependence-loop spacing)
States: a1=G2, a2=G1, a3=alpha*v, a4=theta; swap2=gamma, swap3=alpha;
beta/c/1.0 ride ph2 delay lanes from CONST_1/CONST_2/ONE_F32.
Rounding order matches the jax reference exactly (same op/rounding
sequence as the v4 kernel; bit-identical results).
"""

import numpy as np

ALPHA = 0.95
BETA = 0.995
GAMMA = 0.35

B, N, T = 16, 2048, 1000
NCORES = 8
NSH = N // NCORES           # 256 neurons/core
NG = (B * NSH) // 128       # 32 groups of 128 lanes/core

_CACHE = {}

# ---------------------------------------------------------------- schedule --
# Engine-neutral description of the uop program (validated cycle-accurately
# offline against the numpy reference before HW bring-up).

PREV_ALU, CURR_ALU, NEXT_A, NEXT_B, CURR_SWAP = 0, 1, 2, 3, 4
PD0, PD1, PD2, PD3, PD4, PD5 = 5, 6, 7, 8, 9, 10
D_PREV_ALU, D_PREV_DELAY = 0, 5
I_SRC0, I_SRC1, I_C0, I_C1, I_C2, I_ZERO, I_ONE = (
    "SRC0", "SRC1", "C0", "C1", "C2", "ZERO", "ONE")


class _Blk:
    def __init__(self, op="NOP", a_src=PREV_ALU, b_src=PREV_ALU, out_en=False,
                 afl=False, bfl=False, swap_en=False, delay=None):
        self.op, self.a_src, self.b_src = op, a_src, b_src
        self.out_en, self.afl, self.bfl, self.swap_en = out_en, afl, bfl, swap_en
        self.delay = delay or {}


class _Uop:
    def __init__(self, name="", req0=False, req1=False, wr_en=False,
                 out_sel="ALU", trig=None, repeat=1):
        self.name, self.req0, self.req1 = name, req0, req1
        self.wr_en, self.out_sel = wr_en, out_sel
        self.trig, self.repeat = trig or [], repeat
        self.inp = {}
        self.blocks = [_Blk() for _ in range(8)]


def _build_sched():
    init0 = _Uop("INIT0", req1=True, trig=[("COUNT", 1)])
    init0.inp = {1: I_SRC1, 2: I_C0, 3: I_ZERO, 4: I_ONE}
    init0.blocks[0] = _Blk(delay={0: D_PREV_DELAY, 1: D_PREV_DELAY,
                                  2: D_PREV_DELAY, 3: D_PREV_DELAY})
    init0.blocks[1] = _Blk("BYPASS", PD3, PD3, afl=True,
                           delay={0: D_PREV_DELAY, 1: D_PREV_DELAY,
                                  2: D_PREV_DELAY, 3: D_PREV_DELAY})
    init0.blocks[2] = _Blk("BYPASS", PD3, PD1, afl=True, swap_en=True,
                           delay={0: D_PREV_DELAY, 2: D_PREV_DELAY,
                                  3: D_PREV_DELAY})
    init0.blocks[3] = _Blk("BYPASS", PD2, PD0, afl=True, swap_en=True,
                           delay={3: D_PREV_DELAY})
    init0.blocks[4] = _Blk("BYPASS", PD3, PD3, afl=True)

    init1 = _Uop("INIT1", trig=[("COUNT", 2)])

    ph0 = _Uop("PH0", req0=True, wr_en=True, out_sel="D2", trig=[("COUNT", 3)])
    ph0.inp = {0: I_SRC0}
    ph0.blocks[0] = _Blk("MUL", PREV_ALU, NEXT_A, out_en=True)
    ph0.blocks[1] = _Blk("MUL", PREV_ALU, NEXT_A, out_en=True)
    ph0.blocks[2] = _Blk("ADD", PREV_ALU, NEXT_A, out_en=True)
    ph0.blocks[3] = _Blk("IS_GE", PREV_ALU, NEXT_A, out_en=True, bfl=True)
    ph0.blocks[4] = _Blk(delay={2: D_PREV_ALU})
    ph0.blocks[5] = _Blk(delay={2: D_PREV_DELAY})
    ph0.blocks[6] = _Blk(delay={2: D_PREV_DELAY})
    ph0.blocks[7] = _Blk(delay={2: D_PREV_DELAY})

    ph1 = _Uop("PH1", wr_en=True, out_sel="ALU", trig=[("COUNT", 4)])
    ph1.blocks[3] = _Blk("MUL", CURR_ALU, NEXT_A, out_en=True,
                         delay={0: D_PREV_ALU})
    ph1.blocks[4] = _Blk("SUB", PD0, PREV_ALU, out_en=True, bfl=True)
    ph1.blocks[5] = _Blk("BYPASS", PREV_ALU, PREV_ALU, out_en=True)
    ph1.blocks[6] = _Blk("BYPASS", PREV_ALU, PREV_ALU, out_en=True)
    ph1.blocks[7] = _Blk("BYPASS", PREV_ALU, PREV_ALU, out_en=True)

    ph2 = _Uop("PH2", trig=[("COUNT", 5)])
    ph2.inp = {2: I_ONE, 5: I_C1, 6: I_C2}
    ph2.blocks[0] = _Blk(delay={1: D_PREV_DELAY, 4: D_PREV_DELAY,
                                5: D_PREV_DELAY})
    ph2.blocks[1] = _Blk("BYPASS", NEXT_A, NEXT_A, afl=True,
                         delay={1: D_PREV_DELAY, 4: D_PREV_DELAY,
                                5: D_PREV_DELAY})
    ph2.blocks[2] = _Blk("SUB", PD1, NEXT_B, afl=True,
                         delay={4: D_PREV_DELAY, 5: D_PREV_DELAY})
    ph2.blocks[3] = _Blk("MUL", NEXT_A, PD4, out_en=True,
                         delay={5: D_PREV_DELAY})
    ph2.blocks[4] = _Blk("ADD", PREV_ALU, PD5, out_en=True)

    ph3 = _Uop("PH3", trig=[("COUNT", 6)])
    ph3.blocks[2] = _Blk("MUL", NEXT_B, CURR_SWAP, out_en=True)
    ph3.blocks[3] = _Blk("MUL", NEXT_B, CURR_SWAP, afl=True,
                         delay={3: D_PREV_ALU})
    ph3.blocks[4] = _Blk("ADD", CURR_ALU, PD3, afl=True)

    ph4 = _Uop("PH4", trig=[("SRC_DONE", "IDLE"), ("COUNT", 2)])

    return [init0, init1, ph0, ph1, ph2, ph3, ph4]


# ------------------------------------------------------------- registration --

def _register_op():
    import concourse.dve_ops as dvo
    from concourse.dve_spec import Spec, Src0, Src1, C0, C1, C2
    from concourse.dve_uop import (
        UopConfig, UopDpConfig, DveOpSpec, AluOp, AluInp, DelayInp, InpSel,
        OutSel, OutPath, Trigger, ENABLE, DISABLE,
    )

    name = "LIF_SCAN"
    if name in dvo._SUB_OPCODE_FOR_NAME:
        return next(o for o in dvo.OPS if o.name == name)

    alu = {"BYPASS": AluOp.BYPASS, "ADD": AluOp.ADD, "SUB": AluOp.SUBTRACT,
           "MUL": AluOp.MULTIPLY, "IS_GE": AluOp.IS_GE}
    ain = {PREV_ALU: AluInp.PREV_ALU_OUT, CURR_ALU: AluInp.CURR_ALU_OUT,
           NEXT_A: AluInp.NEXT_ALU_OUT_A, NEXT_B: AluInp.NEXT_ALU_OUT_B,
           CURR_SWAP: AluInp.CURR_SWAP_OUT,
           PD0: AluInp.PREV_DELAY_0, PD1: AluInp.PREV_DELAY_1,
           PD2: AluInp.PREV_DELAY_2, PD3: AluInp.PREV_DELAY_3,
           PD4: AluInp.PREV_DELAY_4, PD5: AluInp.PREV_DELAY_5}
    din = {D_PREV_ALU: DelayInp.PREV_ALU_OUT, D_PREV_DELAY: DelayInp.PREV_DELAY}
    isel = {I_SRC0: InpSel.SRC_0, I_SRC1: InpSel.SRC_1, I_C0: InpSel.CONST_0,
            I_C1: InpSel.CONST_1, I_C2: InpSel.CONST_2, I_ZERO: InpSel.ZERO,
            I_ONE: InpSel.ONE_F32}
    osel = {"ALU": OutSel.ALU_OUT, "D2": OutSel.DELAY_2}
    tmap = {"COUNT": Trigger.COUNT, "SRC_DONE": Trigger.SRC_TENSOR_DONE}

    uops = []
    for su in _build_sched():
        u = UopConfig()
        for lane, sel in su.inp.items():
            u.enable_input(isel[sel], lane)
        u.require_inp0 = ENABLE if su.req0 else DISABLE
        u.require_inp1 = ENABLE if su.req1 else DISABLE
        if su.wr_en:
            u.out[OutPath.WR0_LO] = osel[su.out_sel]
            u.out_enable[OutPath.WR0_LO] = ENABLE
        trigs, nexts = [], []
        for kind, nxt in su.trig:
            trigs.append(tmap[kind])
            nexts.append(0 if nxt == "IDLE" else nxt)
        while len(trigs) < 3:
            trigs.append(Trigger.NONE)
            nexts.append(0)
        u.trigger = tuple(trigs)
        u.next_uop = tuple(nexts)
        u.repeat_count = su.repeat
        for k, sb in enumerate(su.blocks):
            d = UopDpConfig()
            if sb.op != "NOP":
                d.op = alu[sb.op]
                d.alu_src0 = ain[sb.a_src]
                d.alu_src1 = ain[sb.b_src]
                d.alu_out_enable = ENABLE if sb.out_en else DISABLE
                d.alu_out_a_enable = ENABLE if sb.afl else DISABLE
                d.alu_out_b_enable = ENABLE if sb.bfl else DISABLE
                d.swap_enable = ENABLE if sb.swap_en else DISABLE
            for lane, src in sb.delay.items():
                d.delay[lane] = din[src]
                d.delay_enable[lane] = ENABLE
            u.datapath_config[k] = d
        uops.append(u)

    opcode = dvo._CUSTOM_DVE_ROW_BASE + len(dvo.OPS)
    spec = Spec(body=(Src0 * C0 + C1 * C2) * Src1, reference=_lif_reference)
    hand = DveOpSpec(name=name, opcode=opcode, uops=uops, rd1_en=True)
    hand.validate("v3")
    op = dvo.DveOp(name, spec, subdim=False, uops_sha={"v3": hand.sha("v3")})
    dvo._SUB_OPCODE_FOR_NAME[name] = opcode
    dvo.OPS.append(op)
    dvo.CUSTOM_DVE_SPECS[name] = spec
    dvo._COMPILE_CACHE[(name, "v3")] = hand
    return op


def _lif_reference(in0, in1, c0, c1, c2):
    """CoreSim reference: in0 [P,T] u; out [P,2T] interleaved (s, v)."""
    F = np.float32
    in0 = np.asarray(in0, F)
    P = in0.shape[0]
    Tn = int(np.prod(in0.shape[1:]))
    u = in0.reshape(P, Tn)
    alpha = (np.asarray(in1, F).reshape(P)[:, None]
             if in1 is not None else np.full((P, 1), 0.95, F))
    gamma = F(np.asarray(c0, F).flat[0] if isinstance(c0, np.ndarray) else c0)
    beta = F(np.asarray(c1, F).flat[0] if isinstance(c1, np.ndarray) else c1)
    c = F(c2)
    v = np.zeros((P, 1), F)
    th = np.ones((P, 1), F)
    g1 = np.ones((P, 1), F)
    g2 = np.ones((P, 1), F)
    out = np.empty((P, Tn, 2), F)
    for t in range(Tn):
        p = (u[:, t:t + 1] * g2).astype(F)
        m = (p * g1).astype(F)
        w = ((alpha * v).astype(F) + m).astype(F)
        s = (w >= th).astype(F)
        q = (s * th).astype(F)
        vn = (w - q).astype(F)
        t1 = (th * beta).astype(F)
        t2 = (t1 + c).astype(F)
        gs = (s * gamma).astype(F)
        th = (t2 + gs).astype(F)
        out[:, t, 0] = s[:, 0]
        out[:, t, 1] = vn[:, 0]
        g2 = g1
        g1 = (F(1.0) - s).astype(F)
        v = vn
    return out.reshape(P, 2 * Tn)


# ------------------------------------------------------------------ kernel --

def _build_nc(c_imm):
    import concourse.bacc as bacc
    import concourse.mybir as mybir
    import concourse.tile as tile

    LIF = _register_op()
    f32 = mybir.dt.float32

    bf16 = mybir.dt.bfloat16
    nc = bacc.Bacc("TRN2", target_bir_lowering=False, num_devices=NCORES)
    u_d = nc.dram_tensor("u", [NG, 128, T], f32, kind="ExternalInput")
    sv_d = nc.dram_tensor("sv", [NG, 128, 2 * T], bf16, kind="ExternalOutput")
    vec = nc.vector

    with tile.TileContext(nc) as tc_ctx:
        with (
            tc_ctx.tile_pool(name="state", bufs=1) as st,
            tc_ctx.tile_pool(name="upool", bufs=NG) as up,
            tc_ctx.tile_pool(name="svpool", bufs=12) as svp,
        ):
            al = st.tile([128, 1], f32, tag="alpha", name="alpha")
            vec.memset(al[:], ALPHA)
            # preload ALL input tiles up front (128KB/partition, fits SBUF):
            # the DVE never waits on an input DMA after the first group.
            uts = []
            for g in range(NG):
                ut = up.tile([128, T], f32, tag="u", name=f"u{g}")
                nc.sync.dma_start(ut[:], u_d[g, :, :])
                uts.append(ut)
            for g in range(NG):
                ut = uts[g]
                svt = svp.tile([128, 2 * T], bf16, tag="sv", name=f"sv{g}")
                vec._custom_dve(
                    LIF, out=svt[:], in0=ut[:], in1=al[:],
                    s0=GAMMA, s1=BETA, imm2=c_imm,
                )
                # split the store across queues to spread DMA bandwidth
                # (2-way: fewer descriptors beat finer splits on HW)
                q = T  # half of 2T
                for j in range(2):
                    nc.sync.dma_start(sv_d[g, :, j * q:(j + 1) * q],
                                      svt[:, j * q:(j + 1) * q])

    nc.compile()
    return nc


def _get_nc(c_imm):
    key = float(c_imm)
    if key not in _CACHE:
        _CACHE[key] = _build_nc(key)
    return _CACHE[key]


def _shard_inputs(u):
    u = np.asarray(u, dtype=np.float32)
    in_maps = []
    for c in range(NCORES):
        lo, hi = c * NSH, (c + 1) * NSH
        uc = np.ascontiguousarray(
            u[:, lo:hi, :].reshape(B * NSH, T).reshape(NG, 128, T))
        in_maps.append({"u": uc})
    return in_maps


def _unshard(res):
    s_full = np.empty((B, N, T), dtype=np.float32)
    v_full = np.empty((B, N, T), dtype=np.float32)
    for c in range(NCORES):
        lo, hi = c * NSH, (c + 1) * NSH
        sv = np.asarray(res[c]["sv"]).astype(np.float32).reshape(B * NSH, T, 2)
        s_full[:, lo:hi, :] = sv[:, :, 0].reshape(B, NSH, T)
        v_full[:, lo:hi, :] = sv[:, :, 1].reshape(B, NSH, T)
    return s_full, v_full


def _host_fallback(u, theta_base):
    """Exact numpy simulation; only used if theta_base is non-uniform."""
    u = np.asarray(u, np.float32)
    b, n, t = u.shape
    tb = np.asarray(theta_base, np.float32)[0, :, 0]
    v = np.zeros((b, n), np.float32)
    theta = np.broadcast_to(tb, (b, n)).astype(np.float32).copy()
    ref = np.zeros((b, n), np.float32)
    c = (tb * np.float32(1.0 - BETA)).astype(np.float32)
    ss = np.empty((b, n, t), np.float32)
    vs = np.empty((b, n, t), np.float32)
    for i in range(t):
        u_eff = np.where(ref > 0, np.float32(0.0), u[:, :, i])
        v = (np.float32(ALPHA) * v + u_eff).astype(np.float32)
        s = (v >= theta).astype(np.float32)
        v = (v - s * theta).astype(np.float32)
        ref = np.where(s > 0, np.float32(2.0),
                       np.maximum(ref - 1.0, 0.0).astype(np.float32))
        theta = ((theta * np.float32(BETA) + c)
                 + np.float32(GAMMA) * s).astype(np.float32)
        ss[:, :, i] = s
        vs[:, :, i] = v
    return ss, vs


def run(u, theta_base, trace=False):
    from concourse.bass_utils import run_bass_kernel_spmd

    tb = np.asarray(theta_base, dtype=np.float32)
    c_imm = float(np.float32(tb.flat[0]) * np.float32(1.0 - BETA))
    nc = _get_nc(c_imm)
    in_maps = _shard_inputs(u)
    res = run_bass_kernel_spmd(nc, in_maps, core_ids=list(range(NCORES)),
                               trace=trace)
    s_full, v_full = _unshard(res.results)
    return (s_full, v_full), res


def kernel(u, theta_base):
    tb = np.asarray(theta_base, dtype=np.float32)
    if not np.all(tb == tb.flat[0]):
        return _host_fallback(u, theta_base)
    (s_full, v_full), _ = run(u, theta_base)
    return s_full, v_full



# revision 8
# speedup vs baseline: 1.0223x; 1.0011x over previous
"""LIF bank (nn_LIFBank_17059610100011) Trainium2 Bass kernel, v5.

The entire per-lane LIF recurrence (T sequential steps) runs INSIDE one
custom DVE instruction per 128-lane group: a hand-written 8-uop FSM
program streams time along the free dimension at 6 cycles/timestep,
keeping state (v, theta, refractory gates) in per-block a-flops and
emitting (s_t, v_t) interleaved. Replaces the v4 kernel's 6 DVE
instructions per timestep (6000 instructions/core -> 32).

Steady ring (one element per 6 cycles), phases = uops PH0..PH5:
  ph0 (consumes u): p=u*G2, m=p*G1, w=m+AV, s=(w>=TH)->b3; emits s
  ph1: q=s*TH, v'=w-q->b4; emits v'
  ph2: G2<-G1 (a1), G1'=1-s (a2), t1=TH*beta, t2=t1+c
  ph3: gs=s*gamma, AV'=v'*alpha (a3), TH'=t2+gs (a4)
  ph4, ph5: bubbles (dependence-loop spacing)
States: a1=G2, a2=G1, a3=alpha*v, a4=theta; swap2=gamma, swap3=alpha;
beta/c/1.0 ride ph2 delay lanes from CONST_1/CONST_2/ONE_F32.
Rounding order matches the jax reference exactly (same op/rounding
sequence as the v4 kernel; bit-identical results).
"""

import numpy as np

ALPHA = 0.95
BETA = 0.995
GAMMA = 0.35

B, N, T = 16, 2048, 1000
NCORES = 8
NSH = N // NCORES           # 256 neurons/core
NG = (B * NSH) // 128       # 32 groups of 128 lanes/core

_CACHE = {}

# ---------------------------------------------------------------- schedule --
# Engine-neutral description of the uop program (validated cycle-accurately
# offline against the numpy reference before HW bring-up).

PREV_ALU, CURR_ALU, NEXT_A, NEXT_B, CURR_SWAP = 0, 1, 2, 3, 4
PD0, PD1, PD2, PD3, PD4, PD5 = 5, 6, 7, 8, 9, 10
D_PREV_ALU, D_PREV_DELAY = 0, 5
I_SRC0, I_SRC1, I_C0, I_C1, I_C2, I_ZERO, I_ONE = (
    "SRC0", "SRC1", "C0", "C1", "C2", "ZERO", "ONE")


class _Blk:
    def __init__(self, op="NOP", a_src=PREV_ALU, b_src=PREV_ALU, out_en=False,
                 afl=False, bfl=False, swap_en=False, delay=None):
        self.op, self.a_src, self.b_src = op, a_src, b_src
        self.out_en, self.afl, self.bfl, self.swap_en = out_en, afl, bfl, swap_en
        self.delay = delay or {}


class _Uop:
    def __init__(self, name="", req0=False, req1=False, wr_en=False,
                 out_sel="ALU", trig=None, repeat=1):
        self.name, self.req0, self.req1 = name, req0, req1
        self.wr_en, self.out_sel = wr_en, out_sel
        self.trig, self.repeat = trig or [], repeat
        self.inp = {}
        self.blocks = [_Blk() for _ in range(8)]


def _build_sched():
    init0 = _Uop("INIT0", req1=True, trig=[("COUNT", 1)])
    init0.inp = {1: I_SRC1, 2: I_C0, 3: I_ZERO, 4: I_ONE}
    init0.blocks[0] = _Blk(delay={0: D_PREV_DELAY, 1: D_PREV_DELAY,
                                  2: D_PREV_DELAY, 3: D_PREV_DELAY})
    init0.blocks[1] = _Blk("BYPASS", PD3, PD3, afl=True,
                           delay={0: D_PREV_DELAY, 1: D_PREV_DELAY,
                                  2: D_PREV_DELAY, 3: D_PREV_DELAY})
    init0.blocks[2] = _Blk("BYPASS", PD3, PD1, afl=True, swap_en=True,
                           delay={0: D_PREV_DELAY, 2: D_PREV_DELAY,
                                  3: D_PREV_DELAY})
    init0.blocks[3] = _Blk("BYPASS", PD2, PD0, afl=True, swap_en=True,
                           delay={3: D_PREV_DELAY})
    init0.blocks[4] = _Blk("BYPASS", PD3, PD3, afl=True)

    init1 = _Uop("INIT1", trig=[("COUNT", 2)])

    ph0 = _Uop("PH0", req0=True, wr_en=True, out_sel="D2", trig=[("COUNT", 3)])
    ph0.inp = {0: I_SRC0}
    ph0.blocks[0] = _Blk("MUL", PREV_ALU, NEXT_A, out_en=True)
    ph0.blocks[1] = _Blk("MUL", PREV_ALU, NEXT_A, out_en=True)
    ph0.blocks[2] = _Blk("ADD", PREV_ALU, NEXT_A, out_en=True)
    ph0.blocks[3] = _Blk("IS_GE", PREV_ALU, NEXT_A, out_en=True, bfl=True)
    ph0.blocks[4] = _Blk(delay={2: D_PREV_ALU})
    ph0.blocks[5] = _Blk(delay={2: D_PREV_DELAY})
    ph0.blocks[6] = _Blk(delay={2: D_PREV_DELAY})
    ph0.blocks[7] = _Blk(delay={2: D_PREV_DELAY})

    ph1 = _Uop("PH1", wr_en=True, out_sel="ALU", trig=[("COUNT", 4)])
    ph1.blocks[3] = _Blk("MUL", CURR_ALU, NEXT_A, out_en=True,
                         delay={0: D_PREV_ALU})
    ph1.blocks[4] = _Blk("SUB", PD0, PREV_ALU, out_en=True, bfl=True)
    ph1.blocks[5] = _Blk("BYPASS", PREV_ALU, PREV_ALU, out_en=True)
    ph1.blocks[6] = _Blk("BYPASS", PREV_ALU, PREV_ALU, out_en=True)
    ph1.blocks[7] = _Blk("BYPASS", PREV_ALU, PREV_ALU, out_en=True)

    ph2 = _Uop("PH2", trig=[("COUNT", 5)])
    ph2.inp = {2: I_ONE, 5: I_C1, 6: I_C2}
    ph2.blocks[0] = _Blk(delay={1: D_PREV_DELAY, 4: D_PREV_DELAY,
                                5: D_PREV_DELAY})
    ph2.blocks[1] = _Blk("BYPASS", NEXT_A, NEXT_A, afl=True,
                         delay={1: D_PREV_DELAY, 4: D_PREV_DELAY,
                                5: D_PREV_DELAY})
    ph2.blocks[2] = _Blk("SUB", PD1, NEXT_B, afl=True,
                         delay={4: D_PREV_DELAY, 5: D_PREV_DELAY})
    ph2.blocks[3] = _Blk("MUL", NEXT_A, PD4, out_en=True,
                         delay={5: D_PREV_DELAY})
    ph2.blocks[4] = _Blk("ADD", PREV_ALU, PD5, out_en=True)

    ph3 = _Uop("PH3", trig=[("COUNT", 6)])
    ph3.blocks[2] = _Blk("MUL", NEXT_B, CURR_SWAP, out_en=True)
    ph3.blocks[3] = _Blk("MUL", NEXT_B, CURR_SWAP, afl=True,
                         delay={3: D_PREV_ALU})
    ph3.blocks[4] = _Blk("ADD", CURR_ALU, PD3, afl=True)

    ph4 = _Uop("PH4", trig=[("SRC_DONE", "IDLE"), ("COUNT", 2)])

    return [init0, init1, ph0, ph1, ph2, ph3, ph4]


# ------------------------------------------------------------- registration --

def _register_op():
    import concourse.dve_ops as dvo
    from concourse.dve_spec import Spec, Src0, Src1, C0, C1, C2
    from concourse.dve_uop import (
        UopConfig, UopDpConfig, DveOpSpec, AluOp, AluInp, DelayInp, InpSel,
        OutSel, OutPath, Trigger, ENABLE, DISABLE,
    )

    name = "LIF_SCAN"
    if name in dvo._SUB_OPCODE_FOR_NAME:
        return next(o for o in dvo.OPS if o.name == name)

    alu = {"BYPASS": AluOp.BYPASS, "ADD": AluOp.ADD, "SUB": AluOp.SUBTRACT,
           "MUL": AluOp.MULTIPLY, "IS_GE": AluOp.IS_GE}
    ain = {PREV_ALU: AluInp.PREV_ALU_OUT, CURR_ALU: AluInp.CURR_ALU_OUT,
           NEXT_A: AluInp.NEXT_ALU_OUT_A, NEXT_B: AluInp.NEXT_ALU_OUT_B,
           CURR_SWAP: AluInp.CURR_SWAP_OUT,
           PD0: AluInp.PREV_DELAY_0, PD1: AluInp.PREV_DELAY_1,
           PD2: AluInp.PREV_DELAY_2, PD3: AluInp.PREV_DELAY_3,
           PD4: AluInp.PREV_DELAY_4, PD5: AluInp.PREV_DELAY_5}
    din = {D_PREV_ALU: DelayInp.PREV_ALU_OUT, D_PREV_DELAY: DelayInp.PREV_DELAY}
    isel = {I_SRC0: InpSel.SRC_0, I_SRC1: InpSel.SRC_1, I_C0: InpSel.CONST_0,
            I_C1: InpSel.CONST_1, I_C2: InpSel.CONST_2, I_ZERO: InpSel.ZERO,
            I_ONE: InpSel.ONE_F32}
    osel = {"ALU": OutSel.ALU_OUT, "D2": OutSel.DELAY_2}
    tmap = {"COUNT": Trigger.COUNT, "SRC_DONE": Trigger.SRC_TENSOR_DONE}

    uops = []
    for su in _build_sched():
        u = UopConfig()
        for lane, sel in su.inp.items():
            u.enable_input(isel[sel], lane)
        u.require_inp0 = ENABLE if su.req0 else DISABLE
        u.require_inp1 = ENABLE if su.req1 else DISABLE
        if su.wr_en:
            u.out[OutPath.WR0_LO] = osel[su.out_sel]
            u.out_enable[OutPath.WR0_LO] = ENABLE
        trigs, nexts = [], []
        for kind, nxt in su.trig:
            trigs.append(tmap[kind])
            nexts.append(0 if nxt == "IDLE" else nxt)
        while len(trigs) < 3:
            trigs.append(Trigger.NONE)
            nexts.append(0)
        u.trigger = tuple(trigs)
        u.next_uop = tuple(nexts)
        u.repeat_count = su.repeat
        for k, sb in enumerate(su.blocks):
            d = UopDpConfig()
            if sb.op != "NOP":
                d.op = alu[sb.op]
                d.alu_src0 = ain[sb.a_src]
                d.alu_src1 = ain[sb.b_src]
                d.alu_out_enable = ENABLE if sb.out_en else DISABLE
                d.alu_out_a_enable = ENABLE if sb.afl else DISABLE
                d.alu_out_b_enable = ENABLE if sb.bfl else DISABLE
                d.swap_enable = ENABLE if sb.swap_en else DISABLE
            for lane, src in sb.delay.items():
                d.delay[lane] = din[src]
                d.delay_enable[lane] = ENABLE
            u.datapath_config[k] = d
        uops.append(u)

    opcode = dvo._CUSTOM_DVE_ROW_BASE + len(dvo.OPS)
    spec = Spec(body=(Src0 * C0 + C1 * C2) * Src1, reference=_lif_reference)
    hand = DveOpSpec(name=name, opcode=opcode, uops=uops, rd1_en=True)
    hand.validate("v3")
    op = dvo.DveOp(name, spec, subdim=False, uops_sha={"v3": hand.sha("v3")})
    dvo._SUB_OPCODE_FOR_NAME[name] = opcode
    dvo.OPS.append(op)
    dvo.CUSTOM_DVE_SPECS[name] = spec
    dvo._COMPILE_CACHE[(name, "v3")] = hand
    return op


def _lif_reference(in0, in1, c0, c1, c2):
    """CoreSim reference: in0 [P,T] u; out [P,2T] interleaved (s, v)."""
    F = np.float32
    in0 = np.asarray(in0, F)
    P = in0.shape[0]
    Tn = int(np.prod(in0.shape[1:]))
    u = in0.reshape(P, Tn)
    alpha = (np.asarray(in1, F).reshape(P)[:, None]
             if in1 is not None else np.full((P, 1), 0.95, F))
    gamma = F(np.asarray(c0, F).flat[0] if isinstance(c0, np.ndarray) else c0)
    beta = F(np.asarray(c1, F).flat[0] if isinstance(c1, np.ndarray) else c1)
    c = F(c2)
    v = np.zeros((P, 1), F)
    th = np.ones((P, 1), F)
    g1 = np.ones((P, 1), F)
    g2 = np.ones((P, 1), F)
    out = np.empty((P, Tn, 2), F)
    for t in range(Tn):
        p = (u[:, t:t + 1] * g2).astype(F)
        m = (p * g1).astype(F)
        w = ((alpha * v).astype(F) + m).astype(F)
        s = (w >= th).astype(F)
        q = (s * th).astype(F)
        vn = (w - q).astype(F)
        t1 = (th * beta).astype(F)
        t2 = (t1 + c).astype(F)
        gs = (s * gamma).astype(F)
        th = (t2 + gs).astype(F)
        out[:, t, 0] = s[:, 0]
        out[:, t, 1] = vn[:, 0]
        g2 = g1
        g1 = (F(1.0) - s).astype(F)
        v = vn
    return out.reshape(P, 2 * Tn)


# ------------------------------------------------------------------ kernel --

def _build_nc(c_imm):
    import concourse.bacc as bacc
    import concourse.mybir as mybir
    import concourse.tile as tile

    LIF = _register_op()
    f32 = mybir.dt.float32

    bf16 = mybir.dt.bfloat16
    nc = bacc.Bacc("TRN2", target_bir_lowering=False, num_devices=NCORES)
    u_d = nc.dram_tensor("u", [NG, 128, T], f32, kind="ExternalInput")
    sv_d = nc.dram_tensor("sv", [NG, 128, 2 * T], bf16, kind="ExternalOutput")
    vec = nc.vector

    with tile.TileContext(nc) as tc_ctx:
        with (
            tc_ctx.tile_pool(name="state", bufs=1) as st,
            tc_ctx.tile_pool(name="upool", bufs=NG) as up,
            tc_ctx.tile_pool(name="svpool", bufs=12) as svp,
        ):
            al = st.tile([128, 1], f32, tag="alpha", name="alpha")
            vec.memset(al[:], ALPHA)
            # preload ALL input tiles up front (128KB/partition, fits SBUF):
            # the DVE never waits on an input DMA after the first group.
            uts = []
            for g in range(NG):
                ut = up.tile([128, T], f32, tag="u", name=f"u{g}")
                nc.sync.dma_start(ut[:], u_d[g, :, :])
                uts.append(ut)
            for g in range(NG):
                ut = uts[g]
                svt = svp.tile([128, 2 * T], bf16, tag="sv", name=f"sv{g}")
                vec._custom_dve(
                    LIF, out=svt[:], in0=ut[:], in1=al[:],
                    s0=GAMMA, s1=BETA, imm2=c_imm,
                )
                # single store descriptor per group: measured monotone win
                # from coarser store splits (128 desc 185.1us -> 64 desc
                # 183.8us); descriptor overhead dominates queue parallelism
                nc.sync.dma_start(sv_d[g, :, :], svt[:, :])

    nc.compile()
    return nc


def _get_nc(c_imm):
    key = float(c_imm)
    if key not in _CACHE:
        _CACHE[key] = _build_nc(key)
    return _CACHE[key]


def _shard_inputs(u):
    u = np.asarray(u, dtype=np.float32)
    in_maps = []
    for c in range(NCORES):
        lo, hi = c * NSH, (c + 1) * NSH
        uc = np.ascontiguousarray(
            u[:, lo:hi, :].reshape(B * NSH, T).reshape(NG, 128, T))
        in_maps.append({"u": uc})
    return in_maps


def _unshard(res):
    s_full = np.empty((B, N, T), dtype=np.float32)
    v_full = np.empty((B, N, T), dtype=np.float32)
    for c in range(NCORES):
        lo, hi = c * NSH, (c + 1) * NSH
        sv = np.asarray(res[c]["sv"]).astype(np.float32).reshape(B * NSH, T, 2)
        s_full[:, lo:hi, :] = sv[:, :, 0].reshape(B, NSH, T)
        v_full[:, lo:hi, :] = sv[:, :, 1].reshape(B, NSH, T)
    return s_full, v_full


def _host_fallback(u, theta_base):
    """Exact numpy simulation; only used if theta_base is non-uniform."""
    u = np.asarray(u, np.float32)
    b, n, t = u.shape
    tb = np.asarray(theta_base, np.float32)[0, :, 0]
    v = np.zeros((b, n), np.float32)
    theta = np.broadcast_to(tb, (b, n)).astype(np.float32).copy()
    ref = np.zeros((b, n), np.float32)
    c = (tb * np.float32(1.0 - BETA)).astype(np.float32)
    ss = np.empty((b, n, t), np.float32)
    vs = np.empty((b, n, t), np.float32)
    for i in range(t):
        u_eff = np.where(ref > 0, np.float32(0.0), u[:, :, i])
        v = (np.float32(ALPHA) * v + u_eff).astype(np.float32)
        s = (v >= theta).astype(np.float32)
        v = (v - s * theta).astype(np.float32)
        ref = np.where(s > 0, np.float32(2.0),
                       np.maximum(ref - 1.0, 0.0).astype(np.float32))
        theta = ((theta * np.float32(BETA) + c)
                 + np.float32(GAMMA) * s).astype(np.float32)
        ss[:, :, i] = s
        vs[:, :, i] = v
    return ss, vs


def run(u, theta_base, trace=False):
    from concourse.bass_utils import run_bass_kernel_spmd

    tb = np.asarray(theta_base, dtype=np.float32)
    c_imm = float(np.float32(tb.flat[0]) * np.float32(1.0 - BETA))
    nc = _get_nc(c_imm)
    in_maps = _shard_inputs(u)
    res = run_bass_kernel_spmd(nc, in_maps, core_ids=list(range(NCORES)),
                               trace=trace)
    s_full, v_full = _unshard(res.results)
    return (s_full, v_full), res


def kernel(u, theta_base):
    tb = np.asarray(theta_base, dtype=np.float32)
    if not np.all(tb == tb.flat[0]):
        return _host_fallback(u, theta_base)
    (s_full, v_full), _ = run(u, theta_base)
    return s_full, v_full

